# revision 1
# baseline (speedup 1.0000x reference)
"""Angular-select masked-FFT kernel for Trainium2 (8 NeuronCores, data-parallel over batch).

Math: per (b, g): diff[w] = sum_h ||re0|-|im1|| + ||re1|-|im0||; select 64 smallest w;
out = fft_w(ifft_h(x masked to sel columns)) + 0.5, emitted as interleaved re/im f32.

Device algorithm per (b, g) unit, software-pipelined (phase2 lags phase1 by 4 units):
  phase 1 (select):
    - diff: one [128, 4x512] in-place elementwise chain (ACT/DVE/gpsimd), then
      PE ones-matmul reduces h -> PSUM [1, 512] diff row.
    - rank[w] = #{w': d[w'] < d[w]} via fused is_lt+accum DVE ops against the
      partition-broadcast row; selected = rank < 64 (exactly 64, no ties).
    - sparse_gather compacts selected w ids; int16 idx lists wrap position i -> [i%16, i//16].
    - one dma_gather pulls all 4 tensors' selected columns (256 rows of the stacked
      host-transposed bf16 copy) in stage-B lhsT layout; one more pulls the matching
      rows of the stacked interleaved DFT const matrices (+ bias rows folding +0.5 in).
  phase 2 (FFTs as matmuls):
    - stage B (ifft over h): Y^T[t, h'] += lhsT-gathered X vs IDFT consts (PE, bf16).
    - stage C (fft over w): interleaved-output matmuls Y^T @ C1/C2 -> PSUM [128, (w,2)]
      -> evict (DVE/ACT alternating) -> contiguous DMA to DRAM.
"""

import os
import sys
from contextlib import ExitStack

import numpy as np

sys.path.insert(0, "/opt/trn_rl_repo")

B, C, H, W = 32, 4, 512, 512
T = 64
G = 2
NCORES = 8
BPC = B // NCORES  # samples per core

_cache = {}


def _build_consts():
    import ml_dtypes

    h = np.arange(H, dtype=np.float64)
    th = 2.0 * np.pi * np.outer(h, h) / H
    cos_i = (np.cos(th) / H).astype(np.float32)
    sin_i = (np.sin(th) / H).astype(np.float32)
    w = np.arange(W, dtype=np.float64)
    tw = 2.0 * np.pi * np.outer(w, w) / W
    fr = np.cos(tw).astype(np.float32)
    fi = (-np.sin(tw)).astype(np.float32)
    # c1: coeff of Yr -> (re, im) interleaved; c2: coeff of Yi -> (re, im)
    c1 = np.zeros((W + 1, 2 * W), np.float32)
    c2 = np.zeros((W + 1, 2 * W), np.float32)
    c1[:W, 0::2] = fr
    c1[:W, 1::2] = fi
    c2[:W, 0::2] = -fi
    c2[:W, 1::2] = fr
    c1[W, 0::2] = 0.5  # bias row: +0.5 on real part only
    bf = ml_dtypes.bfloat16
    cboth = np.concatenate([c1, c2], axis=0).astype(bf)  # [1026, 1024]
    return cos_i.astype(bf), sin_i.astype(bf), (-sin_i).astype(bf), cboth


def _build_kernel():
    import concourse.bass as bass
    import concourse.tile as tile
    from concourse import bacc, mybir

    f32 = mybir.dt.float32
    bf16 = mybir.dt.bfloat16
    i16 = mybir.dt.int16
    i32 = mybir.dt.int32
    u32 = mybir.dt.uint32
    u8 = mybir.dt.uint8
    Alu = mybir.AluOpType
    Act = mybir.ActivationFunctionType
    MASK = 0x7FFFFFFF

    nc = bacc.Bacc("TRN2", target_bir_lowering=False, debug=False, num_devices=NCORES)

    xr32 = nc.dram_tensor("xr32", [BPC, C, H, W], f32, kind="ExternalInput").ap()
    xi32 = nc.dram_tensor("xi32", [BPC, C, H, W], f32, kind="ExternalInput").ap()
    # stacked transposed bf16: row ((c*2+ri)*512 + w) -> x[b,c,:,w] (re/im by ri)
    x16 = nc.dram_tensor("x16", [BPC, C * 2 * W, H], bf16, kind="ExternalInput").ap()
    cos_d = nc.dram_tensor("cos_i", [H, H], bf16, kind="ExternalInput").ap()
    sin_d = nc.dram_tensor("sin_i", [H, H], bf16, kind="ExternalInput").ap()
    nsin_d = nc.dram_tensor("nsin_i", [H, H], bf16, kind="ExternalInput").ap()
    cb_d = nc.dram_tensor("cboth", [2 * (W + 1), 2 * W], bf16, kind="ExternalInput").ap()
    out_d = nc.dram_tensor("out", [BPC, C, H, W, 2], f32, kind="ExternalOutput").ap()
    dscr = nc.dram_tensor("dscr", [BPC, G, 512], f32).ap()
    vscr = nc.dram_tensor("vscr", [BPC, G, 512], f32).ap()

    with tile.TileContext(nc) as tc, ExitStack() as ctx:
        const_pool = ctx.enter_context(tc.tile_pool(name="consts", bufs=1))
        xpool = ctx.enter_context(tc.tile_pool(name="x", bufs=2))
        spool = ctx.enter_context(tc.tile_pool(name="small", bufs=8))
        bpool = ctx.enter_context(tc.tile_pool(name="brow", bufs=3))
        gpool = ctx.enter_context(tc.tile_pool(name="gather", bufs=8))
        ypool = ctx.enter_context(tc.tile_pool(name="y", bufs=4))
        psum_d = ctx.enter_context(tc.tile_pool(name="psd", bufs=2, space="PSUM"))
        psum_y = ctx.enter_context(tc.tile_pool(name="psy", bufs=1, space="PSUM"))
        psum_o = ctx.enter_context(tc.tile_pool(name="pso", bufs=2, space="PSUM"))
        opool = ctx.enter_context(tc.tile_pool(name="ostage", bufs=8))

        # --- constants in SBUF ---
        sb_cos = const_pool.tile([128, 4, H], bf16)
        sb_sin = const_pool.tile([128, 4, H], bf16)
        sb_nsin = const_pool.tile([128, 4, H], bf16)
        nc.sync.dma_start(sb_cos[:], cos_d.rearrange("(a p) w -> p a w", p=128))
        nc.sync.dma_start(sb_sin[:], sin_d.rearrange("(a p) w -> p a w", p=128))
        nc.sync.dma_start(sb_nsin[:], nsin_d.rearrange("(a p) w -> p a w", p=128))
        iota_i = const_pool.tile([128, 4], i32)
        iota_f = const_pool.tile([128, 4], f32)
        nc.gpsimd.iota(iota_i[:], pattern=[[1, 4]], base=0, channel_multiplier=4)
        nc.vector.tensor_copy(iota_f[:], iota_i[:])
        ones_t = const_pool.tile([128, 1], f32)
        nc.vector.memset(ones_t[:], 1.0)

        state = {}

        def phase1(b, g):
            c0, c1c = 2 * g, 2 * g + 1
            # ---- diff[w]: full-width in-place elementwise + PE h-reduce ----
            diff_ps = psum_d.tile([1, W], f32, tag="dps")
            t_re0 = xpool.tile([128, 4, W], f32, tag="re0")
            t_im1 = xpool.tile([128, 4, W], f32, tag="im1")
            t_re1 = xpool.tile([128, 4, W], f32, tag="re1")
            t_im0 = xpool.tile([128, 4, W], f32, tag="im0")
            nc.sync.dma_start(t_re0[:], xr32[b, c0].rearrange("(a p) w -> p a w", p=128))
            nc.sync.dma_start(t_im1[:], xi32[b, c1c].rearrange("(a p) w -> p a w", p=128))
            nc.sync.dma_start(t_re1[:], xr32[b, c1c].rearrange("(a p) w -> p a w", p=128))
            nc.sync.dma_start(t_im0[:], xi32[b, c0].rearrange("(a p) w -> p a w", p=128))
            nc.scalar.activation(t_re0[:], t_re0[:], Act.Abs)
            nc.vector.tensor_scalar(
                t_im1[:].bitcast(i32), t_im1[:].bitcast(i32), MASK, None,
                Alu.bitwise_and,
            )
            nc.scalar.activation(t_re1[:], t_re1[:], Act.Abs)
            nc.vector.tensor_scalar(
                t_im0[:].bitcast(i32), t_im0[:].bitcast(i32), MASK, None,
                Alu.bitwise_and,
            )
            nc.gpsimd.tensor_tensor(t_re0[:], t_re0[:], t_im1[:], Alu.subtract)
            nc.vector.tensor_tensor(t_re1[:], t_re1[:], t_im0[:], Alu.subtract)
            nc.scalar.activation(t_re0[:], t_re0[:], Act.Abs)
            nc.vector.tensor_scalar(
                t_re1[:].bitcast(i32), t_re1[:].bitcast(i32), MASK, None,
                Alu.bitwise_and,
            )
            nc.vector.tensor_tensor(t_re0[:], t_re0[:], t_re1[:], Alu.add)
            for hq in range(4):
                nc.tensor.matmul(
                    diff_ps[:], ones_t[:, 0:1], t_re0[:, hq, :],
                    start=(hq == 0), stop=(hq == 3),
                )
            # ---- rank + select ----
            drow1 = spool.tile([1, W], f32, tag="drow1")
            nc.vector.tensor_copy(drow1[:], diff_ps[:])
            drow = bpool.tile([128, W], f32, tag="drow")
            nc.gpsimd.partition_broadcast(drow[:], drow1[:])
            nc.sync.dma_start(
                dscr[b, g].rearrange("(a b) -> a b", a=1), drow1[0:1, :]
            )
            dcol = spool.tile([128, 4], f32, tag="dcol")
            nc.sync.dma_start(dcol[:], dscr[b, g].rearrange("(a b) -> a b", a=128))
            rank = spool.tile([128, 4], f32, tag="rank")
            cmp = bpool.tile([128, W], f32, tag="cmp")
            for wq in range(4):
                nc.vector.tensor_scalar(
                    cmp[:], drow[:], dcol[:, wq : wq + 1], 0.0, Alu.is_lt,
                    Alu.add, accum_out=rank[:, wq : wq + 1],
                )
            mask = spool.tile([128, 4], u8, tag="mask")
            nc.vector.tensor_scalar(mask[:], rank[:], float(T), None, Alu.is_lt)
            vals = spool.tile([128, 4], f32, tag="vals")
            nc.vector.memset(vals[:], -1.0)
            nc.vector.copy_predicated(vals[:], mask[:], iota_f[:])
            nc.sync.dma_start(vscr[b, g].rearrange("(a b) -> a b", a=128), vals[:])
            v16 = spool.tile([16, 32], f32, tag="v16")
            nc.sync.dma_start(v16[:], vscr[b, g].rearrange("(a b) -> a b", a=16))
            sel_f = spool.tile([16, 4], f32, tag="self")
            nfound = spool.tile([1, 1], u32, tag="nf")
            nc.gpsimd.sparse_gather(sel_f[:], v16[:], num_found=nfound[:])
            sel16 = spool.tile([16, 4], i16, tag="sel16")
            nc.vector.tensor_copy(sel16[:], sel_f[:])
            # ---- index lists + gathers ----
            idx_a = spool.tile([128, 32], i16, tag="idxa")
            nc.vector.memset(idx_a[0:16, 0:16], -1)
            for j in range(4):
                off = (4 * g + j) * W
                nc.vector.tensor_scalar(
                    idx_a[0:16, 4 * j : 4 * j + 4], sel16[:], off, None, Alu.add
                )
            nc.vector.memset(idx_a[0:16, 16:32], 0)
            nc.vector.tensor_copy(idx_a[0:16, 16:20], sel16[:])
            nc.vector.memset(idx_a[0:1, 20:21], W)
            nc.vector.tensor_scalar(
                idx_a[0:16, 24:28], sel16[:], W + 1, None, Alu.add
            )
            nc.vector.memset(idx_a[0:1, 28:29], 2 * W + 1)
            nc.sync.dma_start(idx_a[16:32, :], idx_a[0:16, :])
            nc.sync.dma_start(idx_a[32:64, :], idx_a[0:32, :])
            nc.sync.dma_start(idx_a[64:128, :], idx_a[0:64, :])
            gx = gpool.tile([128, 4, 256], bf16, tag="gx")
            gc = gpool.tile([128, 2, 2 * W], bf16, tag="gc")
            nc.gpsimd.dma_gather(
                gx[:], x16[b], idx_a[:, 0:16], num_idxs=256, num_idxs_reg=256,
                elem_size=H, transpose=True,
            )
            nc.gpsimd.dma_gather(
                gc[:], cb_d[:], idx_a[:, 16:32],
                num_idxs=256, num_idxs_reg=256, elem_size=2 * W,
            )
            state[(b, g)] = (c0, c1c, gx, gc)

        def phase2(b, g):
            c0, c1c, gx, gc = state.pop((b, g))
            for ci, cc in enumerate((c0, c1c)):
                jr, ji = 2 * ci, 2 * ci + 1
                # ---- stage B (ifft over h) ----
                yr_ps = psum_y.tile([64, H], f32, tag="yrp")
                yi_ps = psum_y.tile([64, H], f32, tag="yip")
                for hq in range(4):
                    first = hq == 0
                    last = hq == 3
                    lr = gx[:, hq, 64 * jr : 64 * jr + T]
                    li = gx[:, hq, 64 * ji : 64 * ji + T]
                    nc.tensor.matmul(
                        yr_ps[:], lr, sb_cos[:, hq, :], start=first, stop=False
                    )
                    nc.tensor.matmul(
                        yr_ps[:], li, sb_nsin[:, hq, :], start=False, stop=last
                    )
                    nc.tensor.matmul(
                        yi_ps[:], lr, sb_sin[:, hq, :], start=first, stop=False
                    )
                    nc.tensor.matmul(
                        yi_ps[:], li, sb_cos[:, hq, :], start=False, stop=last
                    )
                yr_sb = ypool.tile([65, H], bf16, tag="yr")
                yi_sb = ypool.tile([65, H], bf16, tag="yi")
                nc.vector.tensor_copy(yr_sb[0:64, :], yr_ps[:])
                nc.vector.tensor_copy(yi_sb[0:64, :], yi_ps[:])
                nc.vector.memset(yr_sb[64:65, :], 1.0)
                nc.vector.memset(yi_sb[64:65, :], 0.0)
                # ---- stage C (fft over w, interleaved out) ----
                for mq in range(4):
                    msl = slice(mq * 128, (mq + 1) * 128)
                    o_ps = psum_o.tile([128, 2 * W], f32, tag="ops")
                    for nh in range(2):
                        nsl = slice(nh * W, (nh + 1) * W)
                        nc.tensor.matmul(
                            o_ps[:, nsl], yr_sb[:, msl], gc[0:65, 0, nsl],
                            start=True, stop=False,
                        )
                        nc.tensor.matmul(
                            o_ps[:, nsl], yi_sb[:, msl], gc[0:65, 1, nsl],
                            start=False, stop=True,
                        )
                    o_sb = opool.tile([128, 2 * W], f32, tag="osb")
                    if mq == 0:
                        nc.vector.tensor_copy(o_sb[:], o_ps[:])
                    else:
                        nc.scalar.mul(o_sb[:], o_ps[:], 1.0)
                    nc.sync.dma_start(
                        out_d[b, cc, msl].rearrange("p a b -> p (a b)"), o_sb[:]
                    )

        units = [(b, g) for b in range(BPC) for g in range(G)]
        LOOKAHEAD = 4
        for k in range(len(units) + LOOKAHEAD):
            if k < len(units):
                phase1(*units[k])
            if k >= LOOKAHEAD:
                phase2(*units[k - LOOKAHEAD])

    nc.compile()
    return nc


def _get_nc():
    if "nc" not in _cache:
        _cache["nc"] = _build_kernel()
    return _cache["nc"]


def _make_in_maps(xr, xi):
    import ml_dtypes

    bf = ml_dtypes.bfloat16
    cos_i, sin_i, nsin_i, cboth = _cache.setdefault("consts", _build_consts())
    # stacked transposed bf16 copy: [B, C, 2, W, H] -> rows (c*2+ri)*W + w
    x16 = np.stack(
        [xr.transpose(0, 1, 3, 2), xi.transpose(0, 1, 3, 2)], axis=2
    ).astype(bf)
    x16 = np.ascontiguousarray(x16).reshape(B, C * 2 * W, H)
    in_maps = []
    for i in range(NCORES):
        sl = slice(i * BPC, (i + 1) * BPC)
        in_maps.append(
            {
                "xr32": xr[sl],
                "xi32": xi[sl],
                "x16": x16[sl],
                "cos_i": cos_i,
                "sin_i": sin_i,
                "nsin_i": nsin_i,
                "cboth": cboth,
            }
        )
    return in_maps


def kernel(x_real: np.ndarray, x_imag: np.ndarray) -> np.ndarray:
    from concourse.bass_utils import run_bass_kernel_spmd

    xr = np.ascontiguousarray(np.asarray(x_real, dtype=np.float32))
    xi = np.ascontiguousarray(np.asarray(x_imag, dtype=np.float32))
    nc = _get_nc()
    in_maps = _make_in_maps(xr, xi)
    res = run_bass_kernel_spmd(nc, in_maps, core_ids=list(range(NCORES)))
    outs = [res.results[i]["out"] for i in range(NCORES)]
    return np.concatenate(outs, axis=0)


if __name__ == "__main__":
    rng = np.random.RandomState(0)
    out = kernel(
        rng.randn(B, C, H, W).astype(np.float32),
        rng.randn(B, C, H, W).astype(np.float32),
    )
    print(out.shape, out.dtype)



# revision 63
# speedup vs baseline: 1.5838x; 1.5838x over previous
"""Angular-select masked-FFT kernel for Trainium2 (8 NeuronCores, data-parallel over batch).

Math: per (b, g): diff[w] = sum_h ||re0|-|im1|| + ||re1|-|im0||; select 64 smallest w;
out = fft_w(ifft_h(x masked to sel columns)) + 0.5, emitted as interleaved re/im f32.

v2 redesign vs baseline: all select-path data movement stays on-chip (no DRAM
round-trips), diff is computed in a host-transposed layout (partition = w) so the
h-reduction is a DVE/ACT free-axis reduce instead of PE ones-matmuls, the top-64
compaction uses a rank -> one-hot -> tiny-matmul scatter (no sparse_gather, no
vscr), stage B fuses re/im into M=128 matmuls with DVE cross-partition combines,
and stage C runs K=128 single-stream matmuls with the +0.5 bias folded into the
PSUM eviction.

Per (b, g) unit:
  phase 1 (select):
    - 4 transposed f32 loads [128p(w%128), 4a(w//128), 512h]; abs/sub fused via
      abs_max scalar_tensor_tensor; h-sums via tensor_reduce(|.|) on DVE and
      activation(Abs, accum_out) quarters on ACT -> dcol [128, 4].
    - drow via 4 PE transpose-matmuls -> [1, 512] -> gpsimd partition_broadcast.
    - rank[w] = #{d[w'] < d[w]} via 4 fused is_lt+accum DVE ops; one-hot slabs
      (is_equal, mult) place each selected w at free-position rank; 16 tiny PE
      matmuls reduce slabs -> w16 [16, 4] = compacted w list (wrapped for gather).
    - idx lists built on 16 partitions, replicated to 128 via one tiled-eye
      matmul; two dma_gathers pull x columns (stage-B lhsT layout) and the
      stacked interleaved DFT const rows (c1@sel ; c2@sel -> K=128).
  phase 2 (FFTs as matmuls):
    - stage B: A = [lr|li]@cos, B = [lr|li]@sin (M=128); Yr = A[0:64]-B[64:128],
      Yi = B[0:64]+A[64:128] combined straight into bf16 stage-C lhsT tiles.
    - stage C: per (ch, mq, nh) one K=128 matmul vs gathered interleaved consts;
      eviction adds +0.5 to real lanes (ACT strided pair / DVE / GPS biasrow).
"""

import os
import sys
from contextlib import ExitStack

import numpy as np

sys.path.insert(0, "/opt/trn_rl_repo")

B, C, H, W = 32, 4, 512, 512
T = 64
G = 2
NCORES = 8
BPC = B // NCORES  # samples per core

_cache = {}


def _build_consts():
    import ml_dtypes

    bf = ml_dtypes.bfloat16
    h = np.arange(H, dtype=np.float64)
    th = 2.0 * np.pi * np.outer(h, h) / H
    cos_i = (np.cos(th) / H).astype(np.float32)
    sin_i = (np.sin(th) / H).astype(np.float32)
    w = np.arange(W, dtype=np.float64)
    tw = 2.0 * np.pi * np.outer(w, w) / W
    fr = np.cos(tw).astype(np.float32)
    fi = (-np.sin(tw)).astype(np.float32)
    # c1: coeff of Yr -> (re, im) interleaved; c2: coeff of Yi -> (re, im)
    c1 = np.zeros((W + 1, 2 * W), np.float32)
    c2 = np.zeros((W + 1, 2 * W), np.float32)
    c1[:W, 0::2] = fr
    c1[:W, 1::2] = fi
    c2[:W, 0::2] = -fi
    c2[:W, 1::2] = fr
    cboth = np.concatenate([c1, c2], axis=0).astype(bf)  # [1026, 1024]
    ident128 = np.eye(128, dtype=np.float32)
    tiled_eye16 = np.tile(np.eye(16, dtype=np.float32), (1, 8))  # [16, 128]
    iota64row = np.tile(np.arange(64, dtype=np.float32), (128, 1))  # [128, 64]
    iotaw = (np.arange(4, dtype=np.float32)[None, :] * 128
             + np.arange(128, dtype=np.float32)[:, None])  # [128, 4] = 128a+p
    biasrow = np.zeros((128, 2 * W), np.float32)
    biasrow[:, 0::2] = 0.5
    return (cos_i.astype(bf), sin_i.astype(bf), cboth, ident128,
            tiled_eye16, iota64row, iotaw, biasrow)


def _build_kernel():
    import concourse.bass as bass
    import concourse.tile as tile
    from concourse import bacc, mybir

    f32 = mybir.dt.float32
    bf16 = mybir.dt.bfloat16
    i16 = mybir.dt.int16
    i32 = mybir.dt.int32
    Alu = mybir.AluOpType
    Act = mybir.ActivationFunctionType
    MASK = 0x7FFFFFFF

    nc = bacc.Bacc("TRN2", target_bir_lowering=False, debug=False, num_devices=NCORES)

    # transposed f32 input: [b, g, p(w%128), t(re0,im1,re1,im0), a(w//128), h]
    # (p outermost: each partition's 32KB is one contiguous DRAM run)
    xt32 = nc.dram_tensor("xt32", [BPC, G, 128, 4, 4, H], f32, kind="ExternalInput").ap()
    # stacked transposed bf16: row ((c*2+ri)*512 + w) -> x[b,c,:,w] (re/im by ri)
    x16 = nc.dram_tensor("x16", [BPC, C * 2 * W, H], bf16, kind="ExternalInput").ap()
    cos_d = nc.dram_tensor("cos_i", [H, H], bf16, kind="ExternalInput").ap()
    sin_d = nc.dram_tensor("sin_i", [H, H], bf16, kind="ExternalInput").ap()
    cb_d = nc.dram_tensor("cboth", [2 * (W + 1), 2 * W], bf16, kind="ExternalInput").ap()
    id_d = nc.dram_tensor("ident128", [128, 128], f32, kind="ExternalInput").ap()
    te_d = nc.dram_tensor("teye16", [16, 128], f32, kind="ExternalInput").ap()
    i64_d = nc.dram_tensor("iota64", [128, 64], f32, kind="ExternalInput").ap()
    iw_d = nc.dram_tensor("iotaw", [128, 4], f32, kind="ExternalInput").ap()
    br_d = nc.dram_tensor("biasrow", [128, 2 * W], f32, kind="ExternalInput").ap()
    out_d = nc.dram_tensor("out", [BPC, C, H, W, 2], f32, kind="ExternalOutput").ap()

    with tile.TileContext(nc) as tc, ExitStack() as ctx:
        const_pool = ctx.enter_context(tc.tile_pool(name="consts", bufs=1))
        xpool = ctx.enter_context(tc.tile_pool(name="x", bufs=3))
        spool = ctx.enter_context(tc.tile_pool(name="small", bufs=6))
        bpool = ctx.enter_context(tc.tile_pool(name="brow", bufs=2))
        gpool = ctx.enter_context(tc.tile_pool(name="gather", bufs=4))
        ypool = ctx.enter_context(tc.tile_pool(name="y", bufs=2))
        psum_s1 = ctx.enter_context(tc.tile_pool(name="ps1", bufs=1, space="PSUM"))
        psum_s2 = ctx.enter_context(tc.tile_pool(name="ps2", bufs=1, space="PSUM"))
        psum_y = ctx.enter_context(tc.tile_pool(name="psy", bufs=1, space="PSUM"))
        psum_o = ctx.enter_context(tc.tile_pool(name="pso", bufs=2, space="PSUM"))
        opool = ctx.enter_context(tc.tile_pool(name="ostage", bufs=2))

        # --- constants in SBUF ---
        sb_cos = const_pool.tile([128, 4, H], bf16)
        sb_sin = const_pool.tile([128, 4, H], bf16)
        nc.sync.dma_start(sb_cos[:], cos_d.rearrange("(a p) w -> p a w", p=128))
        nc.sync.dma_start(sb_sin[:], sin_d.rearrange("(a p) w -> p a w", p=128))
        sb_id = const_pool.tile([128, 128], f32)
        nc.sync.dma_start(sb_id[:], id_d)
        sb_te = const_pool.tile([16, 128], f32)
        nc.sync.dma_start(sb_te[:], te_d)
        sb_i64 = const_pool.tile([128, 64], f32)
        nc.sync.dma_start(sb_i64[:], i64_d)
        sb_iw = const_pool.tile([128, 4], f32)
        nc.sync.dma_start(sb_iw[:], iw_d)
        sb_br = const_pool.tile([128, 2 * W], f32)
        nc.sync.dma_start(sb_br[:], br_d)
        ones_t = const_pool.tile([128, 1], f32)
        nc.vector.memset(ones_t[:], 1.0)

        state = {}
        state_tx = {}

        def load(b, g):
            # ---- loads: [p(w%128), t, a(w//128), h]; 4KB descriptors spread
            #      across 8 queues (big contiguous descriptors stripe poorly) ----
            tx = xpool.tile([128, 4, 4, H], f32, tag="tx")
            for t in range(4):
                nc.sync.dma_start(tx[:, t, 0:2], xt32[b, g, :, t, 0:2])
                nc.sync.dma_start(tx[:, t, 2:4], xt32[b, g, :, t, 2:4])
            state_tx[(b, g)] = tx

        def phase1a(b, g):
            tx = state_tx.pop((b, g))
            # ---- d1 = |t0| - |t1| (-> tx[:,0]), d2 = |t2| - |t3| (-> tx[:,2]) ----
            # (gpsimd is slow and on the critical path: keep it to
            #  broadcast+gathers only; abs on ACT, and/sub on DVE)
            nc.scalar.activation(tx[:, 1], tx[:, 1], Act.Abs)
            nc.scalar.activation(tx[:, 0], tx[:, 0], Act.Abs)
            nc.scalar.activation(tx[:, 2], tx[:, 2], Act.Abs)
            nc.vector.tensor_scalar(
                tx[:, 3].bitcast(i32), tx[:, 3].bitcast(i32), MASK, None,
                Alu.bitwise_and,
            )
            nc.vector.tensor_tensor(tx[:, 2], tx[:, 2], tx[:, 3], Alu.subtract)
            nc.vector.tensor_tensor(tx[:, 0], tx[:, 0], tx[:, 1], Alu.subtract)
            # ---- h-sums of |d| -> dcol [128, 4] (d[w] at [w%128, w//128]) ----
            dc = spool.tile([128, 2, 4], f32, tag="dc")
            nc.vector.tensor_reduce(
                dc[:, 0], tx[:, 0], mybir.AxisListType.X, Alu.add,
                apply_absolute_value=True,
            )
            ascr = bpool.tile([128, H], f32, tag="ascr")
            for a in range(4):
                nc.scalar.activation(
                    ascr[:], tx[:, 2, a], Act.Abs,
                    accum_out=dc[:, 1, a : a + 1],
                )
            dcol = spool.tile([128, 4], f32, tag="dcol")
            nc.vector.tensor_tensor(dcol[:], dc[:, 0], dc[:, 1], Alu.add)
            # ---- drow: 4 PE transposes -> [1, 512] -> broadcast ----
            drow_ps = psum_s1.tile([1, W], f32, tag="drps")
            for a in range(4):
                nc.tensor.transpose(
                    drow_ps[:, 128 * a : 128 * (a + 1)], dcol[:, a : a + 1], sb_id[:]
                )
            drow1 = spool.tile([1, W], f32, tag="drow1")
            nc.vector.tensor_copy(drow1[:], drow_ps[:])
            drow = bpool.tile([128, W], f32, tag="drow")
            nc.gpsimd.partition_broadcast(drow[:], drow1[:])
            state[("a", b, g)] = (dcol, drow)

        def phase1b(b, g):
            dcol, drow = state.pop(("a", b, g))
            # ---- rank + one-hot scatter ----
            rank = spool.tile([128, 4], f32, tag="rank")
            cmp = bpool.tile([128, W], bf16, tag="cmp")
            for a in range(4):
                nc.vector.tensor_scalar(
                    cmp[:], drow[:], dcol[:, a : a + 1], 0.0, Alu.is_lt,
                    Alu.add, accum_out=rank[:, a : a + 1],
                )
            slab = spool.tile([128, 4, 64], f32, tag="slab")
            for a in range(4):
                nc.vector.tensor_scalar(
                    slab[:, a], sb_i64[:], rank[:, a : a + 1],
                    sb_iw[:, a : a + 1], Alu.is_equal, Alu.mult,
                )
            # ---- compact: w16[m, f] = selected w with rank 16f+m ----
            sel_ps = psum_s2.tile([128, 28], f32, tag="selps")
            for f in range(4):
                for a in range(4):
                    nc.tensor.matmul(
                        sel_ps[0:16, f : f + 1],
                        slab[:, a, 16 * f : 16 * (f + 1)],
                        ones_t[:],
                        start=(a == 0), stop=(a == 3),
                    )
            w16 = spool.tile([16, 4], f32, tag="w16")
            nc.vector.tensor_copy(w16[:], sel_ps[0:16, 0:4])
            # ---- idx lists on 16 partitions, then replicate via matmul ----
            idxf = spool.tile([16, 24], f32, tag="idxf")
            for j in range(4):
                nc.vector.tensor_scalar(
                    idxf[:, 4 * j : 4 * (j + 1)], w16[:], float((4 * g + j) * W),
                    None, Alu.add,
                )
            nc.vector.tensor_copy(idxf[:, 16:20], w16[:])
            nc.vector.tensor_scalar(idxf[:, 20:24], w16[:], float(W + 1), None, Alu.add)
            nc.tensor.matmul(
                sel_ps[:, 4:28], sb_te[:], idxf[:], start=True, stop=True
            )
            idx_a = spool.tile([128, 24], i16, tag="idxa")
            nc.vector.tensor_copy(idx_a[:], sel_ps[:, 4:28])
            # ---- gathers ----
            gx = gpool.tile([128, 4, 256], bf16, tag="gx")
            gc = gpool.tile([128, 1, 2 * W], bf16, tag="gc")
            nc.gpsimd.dma_gather(
                gx[:], x16[b], idx_a[:, 0:16], num_idxs=256, num_idxs_reg=256,
                elem_size=H, transpose=True,
            )
            nc.gpsimd.dma_gather(
                gc[:], cb_d[:], idx_a[:, 16:24], num_idxs=128, num_idxs_reg=128,
                elem_size=2 * W,
            )
            state[(b, g)] = (gx, gc)

        def phase2(b, g):
            gx, gc = state.pop((b, g))
            for ci in range(2):
                cc = 2 * g + ci
                csl = slice(128 * ci, 128 * (ci + 1))
                # ---- stage B: A = [lr|li]@cos, B = [lr|li]@sin ----
                a_ps = psum_y.tile([128, H], f32, tag="aps")
                b_ps = psum_y.tile([128, H], f32, tag="bps")
                for hq in range(4):
                    first = hq == 0
                    last = hq == 3
                    nc.tensor.matmul(
                        a_ps[:], gx[:, hq, csl], sb_cos[:, hq, :],
                        start=first, stop=last,
                    )
                    nc.tensor.matmul(
                        b_ps[:], gx[:, hq, csl], sb_sin[:, hq, :],
                        start=first, stop=last,
                    )
                # Yr = A[0:64] - B[64:128]; Yi = B[0:64] + A[64:128]
                # (tensor_tensor allows at most one PSUM operand: stage B
                #  evicts B to SBUF f32 on ACT, then combines PSUM A + SBUF B)
                b_sb = ypool.tile([128, H], f32, tag="bsb")
                nc.scalar.activation(b_sb[:], b_ps[:], Act.Copy)
                y_sb = ypool.tile([128, H], bf16, tag="y")
                nc.vector.tensor_tensor(
                    y_sb[0:64, :], a_ps[0:64, :], b_sb[64:128, :], Alu.subtract
                )
                nc.vector.tensor_tensor(
                    y_sb[64:128, :], a_ps[64:128, :], b_sb[0:64, :], Alu.add
                )
                # ---- stage C: K=128 single-stream + biased eviction ----
                for mq in range(4):
                    msl = slice(mq * 128, (mq + 1) * 128)
                    o_ps = psum_o.tile([128, W, 2], f32, tag="ops")
                    opf = o_ps[:].rearrange("p a b -> p (a b)")
                    for nh in range(2):
                        nsl = slice(nh * W, (nh + 1) * W)
                        nc.tensor.matmul(
                            opf[:, nsl], y_sb[:, msl], gc[0:128, 0, nsl],
                            start=True, stop=True,
                        )
                    o_sb = opool.tile([128, W, 2], f32, tag="osb")
                    osf = o_sb[:].rearrange("p a b -> p (a b)")
                    if mq % 2 == 0:
                        nc.scalar.activation(
                            o_sb[:, :, 0], o_ps[:, :, 0], Act.Copy, bias=0.5
                        )
                        nc.scalar.activation(o_sb[:, :, 1], o_ps[:, :, 1], Act.Copy)
                        dma_eng = nc.scalar
                    else:
                        nc.vector.tensor_tensor(osf[:], opf[:], sb_br[:], Alu.add)
                        dma_eng = nc.sync
                    dma_eng.dma_start(
                        out_d[b, cc, msl].rearrange("p a b -> p (a b)"), osf[:]
                    )

        # 4-stage software pipeline: unit u is loaded at iter u, elementwise-
        # reduced at u+1, selected/gathered at u+2, FFT'd at u+5.  Emission
        # order per iteration keeps each in-order engine queue stocked with
        # ready work ahead of cross-engine waits.
        units = [(b, g) for b in range(BPC) for g in range(G)]
        NU = len(units)
        for k in range(NU + 3):
            if k < NU:
                load(*units[k])
            if 3 <= k < NU + 3:
                phase2(*units[k - 3])
            if 2 <= k < NU + 2:
                phase1b(*units[k - 2])
            if 1 <= k < NU + 1:
                phase1a(*units[k - 1])

    nc.compile()
    return nc


def _get_nc():
    if "nc" not in _cache:
        _cache["nc"] = _build_kernel()
    return _cache["nc"]


def _make_in_maps(xr, xi):
    import ml_dtypes

    bf = ml_dtypes.bfloat16
    (cos_i, sin_i, cboth, ident128, teye16, iota64, iotaw, biasrow) = _cache.setdefault(
        "consts", _build_consts()
    )
    # stacked transposed bf16 copy: [B, C, 2, W, H] -> rows (c*2+ri)*W + w
    x16 = np.stack(
        [xr.transpose(0, 1, 3, 2), xi.transpose(0, 1, 3, 2)], axis=2
    ).astype(bf)
    x16 = np.ascontiguousarray(x16).reshape(B, C * 2 * W, H)
    # transposed f32: [B, G, p(w%128), t(re0,im1,re1,im0), a(w//128), h]
    tens = np.stack(
        [xr[:, 0::2], xi[:, 1::2], xr[:, 1::2], xi[:, 0::2]], axis=2
    )  # [B, G, 4, H, W]
    xt = tens.transpose(0, 1, 2, 4, 3).reshape(B, G, 4, 4, 128, H)
    xt32 = np.ascontiguousarray(xt.transpose(0, 1, 4, 2, 3, 5)).astype(np.float32)
    in_maps = []
    for i in range(NCORES):
        sl = slice(i * BPC, (i + 1) * BPC)
        in_maps.append(
            {
                "xt32": xt32[sl],
                "x16": x16[sl],
                "cos_i": cos_i,
                "sin_i": sin_i,
                "cboth": cboth,
                "ident128": ident128,
                "teye16": teye16,
                "iota64": iota64,
                "iotaw": iotaw,
                "biasrow": biasrow,
            }
        )
    return in_maps


def kernel(x_real: np.ndarray, x_imag: np.ndarray) -> np.ndarray:
    from concourse.bass_utils import run_bass_kernel_spmd

    xr = np.ascontiguousarray(np.asarray(x_real, dtype=np.float32))
    xi = np.ascontiguousarray(np.asarray(x_imag, dtype=np.float32))
    nc = _get_nc()
    in_maps = _make_in_maps(xr, xi)
    res = run_bass_kernel_spmd(nc, in_maps, core_ids=list(range(NCORES)))
    outs = [res.results[i]["out"] for i in range(NCORES)]
    return np.concatenate(outs, axis=0)


if __name__ == "__main__":
    rng = np.random.RandomState(0)
    out = kernel(
        rng.randn(B, C, H, W).astype(np.float32),
        rng.randn(B, C, H, W).astype(np.float32),
    )
    print(out.shape, out.dtype)
